# revision 45
# baseline (speedup 1.0000x reference)
"""Trainium2 Bass kernel for BasicEuclideanDistModel (gnn_message_passing).

Math:
  result = sum_e (beta - ||dz_e + dv_e t_e||)
           - dt * sum_{i<j, s} exp(beta - ||z_i(t_s) - z_j(t_s)||)

Device strategy (8 cores, data parallel):
  * Non-event term: full NxN pairwise distances (halved on host).
    d^2(i,j,s) = F_i(s) . G_j  (K=8 inner product, G time-independent).
    One [8,128]x[8,2048] matmul (fp32r) per (i-tile, s) computes the
    d^2 supertile; DVE relu clamps rounding negatives, ACT computes
    sqrt then exp(-d) with fused row sums.  Each core owns 2 of the
    16 i-tiles, all j, all 10 samples.
  * Event term, split across two independent engines working in
    parallel (events of one u-node always stay together):
    - gpsimd share: d^2(u,v,t) = sum_k A_k(u) B_k(t) C_k(v), a
      14-channel trilinear decomposition with B_k in {1,t,t^2}.
      Events form 8 groups (one per Q7 tile); partition 16g+k holds
      channel k.  ONE ap_gather (SBUF gather, ~27.5ns/idx/core,
      shared index list per group) fetches A_k per segment and C_k
      per event from a [128, N, 2] bf16 channel table; DVE forms
      P = A*C*T (host T = B_k(t)*mask), PE reduces channels with a
      block-ones stationary, one ACT sqrt row-sum -> acc col 20.
    - SWDGE share: baseline scheme -- events grouped by u into
      segments laid out [128, SPD, SLOTD]; dma_gather fetches 256B
      rows (u per segment, v per slot, ~3.8ns/desc aggregate); DVE
      distance algebra, ACT sqrt row-sum -> acc col 21.  Pad slots
      use v=u, t=0 (exactly 0 contribution).
  * beta enters only as a scalar factor / offset -> folded in on host.
  Host combines 8 cores' [128, 24] partial-sum tensors (pure unshard/
  reduction of partials).
"""

import os
import numpy as np


def _import_concourse():
    try:
        import concourse  # noqa: F401
    except ImportError:
        import sys

        for p in ("/opt/trn_rl_repo", "/root/.axon_site/_ro/trn_rl_repo"):
            if os.path.isdir(p) and p not in sys.path:
                sys.path.insert(0, p)


_import_concourse()

from contextlib import ExitStack  # noqa: E402

import concourse.bacc as bacc  # noqa: E402
import concourse.mybir as mybir  # noqa: E402
import concourse.tile as tile  # noqa: E402
from concourse.tile_rust import add_dep_helper  # noqa: E402

N = 2048          # nodes
S = 10            # Riemann samples
NCORES = 8
ITILES = 2        # 128-row i-tiles per core
EV_PER_CORE = 200000 // NCORES       # real events per core

# ---- SWDGE (dma_gather) event layout: slot-major ----
# Events grouped by u-node into segments of <= SLOT_D slots; segments
# sorted by fill count and dealt round-robin to (partition, q) by global
# rank.  One v-side gather op per SLOT POSITION j: since segment fill is
# non-increasing in rank, each op's pad indices form an exact list
# suffix, skipped via num_idxs_reg (no descriptors, no DMA).
SLOT_D = 6        # event slots per segment
SPD = 40          # segments per partition (capacity)
C_EV = SPD * SLOT_D                  # 240 event columns per partition
NSEG = 128 * SPD                     # 5120 segment capacity per core
GELEM = 64        # gather element size in f32 (256B rows)

F32 = mybir.dt.float32
F32R = mybir.dt.float32r
BF16 = mybir.dt.bfloat16
I16 = mybir.dt.int16
AF = mybir.ActivationFunctionType
OP = mybir.AluOpType

_CACHE: dict = {}
_DBG_SPLIT: list = []


def _tt(nc, out, in0, in1, op):
    return nc.vector.tensor_tensor(out, in0, in1, op=op)


def _build(regs=None):
    if "nc" in _CACHE:
        return _CACHE["nc"]
    if regs is None:
        regs = (NSEG, [NSEG] * SLOT_D)
    seg_reg, v_regs = regs

    nc = bacc.Bacc(
        "TRN2", target_bir_lowering=False, debug=False, enable_asserts=False,
        num_swdge_queues=4,
    )

    # inputs coalesced into 2 blobs: per-DMA fixed cost (~2.4us) made 19
    # separate loads a ~46us critical-path prefix
    FB = 228 + 2 * C_EV   # f32: zv(64) zvi(8) tb t2b ident(128) pad(8)
    #                       ev_t(C_EV, slot-major) ev_mask(C_EV)
    IB = (NSEG + SLOT_D * NSEG) // 16
    zv_pad = nc.dram_tensor("zv_pad", [N, GELEM], F32, kind="ExternalInput").ap()
    fblob_d = nc.dram_tensor("fblob", [128, FB], F32, kind="ExternalInput").ap()
    iblob_d = nc.dram_tensor("iblob", [128, IB], I16, kind="ExternalInput").ap()
    out_p = nc.dram_tensor("out_p", [128, 24], F32, kind="ExternalOutput").ap()

    with tile.TileContext(nc) as tc, ExitStack() as ctx:
        cpool = ctx.enter_context(tc.tile_pool(name="const", bufs=1))
        evpool = ctx.enter_context(tc.tile_pool(name="ev", bufs=1))

        # ---------------- input loads (2 coalesced blobs) ----------------
        ib_sb = evpool.tile([128, IB], I16)
        nc.sync.dma_start(ib_sb[:], iblob_d)
        fb_sb = cpool.tile([128, FB], F32)
        nc.sync.dma_start(fb_sb[:], fblob_d)

        UW = NSEG // 16
        u_sb = ib_sb[:, 0:UW]
        v_sb = ib_sb[:, UW:].rearrange("p (a b) -> p a b", a=SLOT_D)
        zv_sb = fb_sb[:, 0:64].rearrange("p (c d) -> p c d", d=4)
        zvi_sb = fb_sb[:, 64:72].rearrange("p (c d) -> p c d", d=4)
        tb = fb_sb[:, 72:82]
        t2b = fb_sb[:, 82:92]
        ident = fb_sb[:, 92:220]
        t_sb = fb_sb[:, 228:228 + C_EV]
        m_sb = fb_sb[:, 228 + C_EV:228 + 2 * C_EV]

        acc = cpool.tile([128, 24], F32)
        nc.vector.memset(acc[:], 0.0)

        # ---------------- event gathers ----------------
        # u-side: one 256B row per segment (rank-major list); v-side: one
        # op per slot position j, same rank-major order, pad suffix
        # skipped via num_idxs_reg.  Skipped slots stay memset-0 and are
        # masked out of d^2.
        d2all = evpool.tile([128, C_EV, 1], F32)
        seg = evpool.tile([128, SPD, GELEM], F32)
        nc.vector.memset(seg[:], 0.0)
        nc.gpsimd.dma_gather(
            seg[:], zv_pad, u_sb, NSEG, seg_reg, GELEM,
            single_packet=False, queue_num=0,
        )
        b_tiles = []
        for j in range(SLOT_D):
            B = evpool.tile([128, SPD, GELEM], F32)
            nc.vector.memset(B[:], 0.0)
            nc.gpsimd.dma_gather(
                B[:], zv_pad, v_sb[:, j, :], NSEG, v_regs[j], GELEM,
                single_packet=False, queue_num=(1 + j) % 4,
            )
            b_tiles.append(B)

        def emit_dma_event_math(j, scratch_pool):
            B = b_tiles[j]
            shape3 = [128, SPD, 1]
            tse = t_sb[:, j * SPD:(j + 1) * SPD].unsqueeze(2)
            mse = m_sb[:, j * SPD:(j + 1) * SPD].unsqueeze(2)

            def sv(d):  # seg channel d
                return seg[:, :, d:d + 1]

            def bv(d):  # B channel d
                return B[:, :, d:d + 1]

            dzx = scratch_pool.tile(shape3, F32, tag="w", name="dzx")
            dvx = scratch_pool.tile(shape3, F32, tag="w", name="dvx")
            dzy = scratch_pool.tile(shape3, F32, tag="w", name="dzy")
            dvy = scratch_pool.tile(shape3, F32, tag="w", name="dvy")
            _tt(nc, dzx[:], sv(0), bv(0), OP.subtract)
            _tt(nc, dvx[:], sv(2), bv(2), OP.subtract)
            _tt(nc, dvx[:], dvx[:], tse, OP.mult)
            _tt(nc, dzx[:], dzx[:], dvx[:], OP.add)          # dx
            _tt(nc, dzy[:], sv(1), bv(1), OP.subtract)
            _tt(nc, dvy[:], sv(3), bv(3), OP.subtract)
            _tt(nc, dvy[:], dvy[:], tse, OP.mult)
            _tt(nc, dzy[:], dzy[:], dvy[:], OP.add)          # dy
            _tt(nc, dzx[:], dzx[:], dzx[:], OP.mult)
            _tt(nc, dzy[:], dzy[:], dzy[:], OP.mult)
            _tt(nc, dzx[:], dzx[:], dzy[:], OP.add)          # d^2
            d2v = d2all[:, j * SPD:(j + 1) * SPD, :]
            _tt(nc, d2v, dzx[:], mse, OP.mult)               # mask pads

        # ---------------- j features  F[p, chunk, 0:8] ----------------
        # [1, a, b, c, zx, vx, zy, vy]; padded to 32 for the PE transpose
        F = cpool.tile([128, 16, 32], F32)
        zx = zv_sb[:, :, 0:1]
        zy = zv_sb[:, :, 1:2]
        vx = zv_sb[:, :, 2:3]
        vy = zv_sb[:, :, 3:4]
        s1 = cpool.tile([128, 16, 1], F32)
        nc.vector.memset(F[:, :, 0:1], 1.0)
        _tt(nc, F[:, :, 1:2], zx, zx, OP.mult)           # a = zx^2 + zy^2
        _tt(nc, s1[:], zy, zy, OP.mult)
        _tt(nc, F[:, :, 1:2], F[:, :, 1:2], s1[:], OP.add)
        s2 = cpool.tile([128, 16, 1], F32)
        _tt(nc, F[:, :, 2:3], zx, vx, OP.mult)           # b = 2(zx vx + zy vy)
        _tt(nc, s2[:], zy, vy, OP.mult)
        _tt(nc, F[:, :, 2:3], F[:, :, 2:3], s2[:], OP.add)
        nc.vector.tensor_scalar_mul(F[:, :, 2:3], F[:, :, 2:3], 2.0)
        s3 = cpool.tile([128, 16, 1], F32)
        _tt(nc, F[:, :, 3:4], vx, vx, OP.mult)           # c = vx^2 + vy^2
        _tt(nc, s3[:], vy, vy, OP.mult)
        _tt(nc, F[:, :, 3:4], F[:, :, 3:4], s3[:], OP.add)
        nc.vector.tensor_copy(F[:, :, 4:5], zx)
        nc.vector.tensor_copy(F[:, :, 5:6], vx)
        nc.vector.tensor_copy(F[:, :, 6:7], zy)
        nc.vector.tensor_copy(F[:, :, 7:8], vy)

        # ---------------- i features  L[p, it, s, 0:8] ----------------
        # [r, 1, t, t^2, -2x, -2tx, -2y, -2ty]
        L = cpool.tile([128, ITILES, S, 32], F32)
        izx = zvi_sb[:, :, 0:1]
        izy = zvi_sb[:, :, 1:2]
        ivx = zvi_sb[:, :, 2:3]
        ivy = zvi_sb[:, :, 3:4]
        ia = cpool.tile([128, ITILES, 1], F32)
        ib = cpool.tile([128, ITILES, 1], F32)
        ic = cpool.tile([128, ITILES, 1], F32)
        s4 = cpool.tile([128, ITILES, 1], F32)
        _tt(nc, ia[:], izx, izx, OP.mult)
        _tt(nc, s4[:], izy, izy, OP.mult)
        _tt(nc, ia[:], ia[:], s4[:], OP.add)
        s5 = cpool.tile([128, ITILES, 1], F32)
        _tt(nc, ib[:], izx, ivx, OP.mult)
        _tt(nc, s5[:], izy, ivy, OP.mult)
        _tt(nc, ib[:], ib[:], s5[:], OP.add)
        nc.vector.tensor_scalar_mul(ib[:], ib[:], 2.0)
        s6 = cpool.tile([128, ITILES, 1], F32)
        _tt(nc, ic[:], ivx, ivx, OP.mult)
        _tt(nc, s6[:], ivy, ivy, OP.mult)
        _tt(nc, ic[:], ic[:], s6[:], OP.add)

        def b_i(v):  # [128, ITILES, 1] -> [128, ITILES, S, 1]
            return v.unsqueeze(2).to_broadcast([128, ITILES, S, 1])

        tv = tb.unsqueeze(1).unsqueeze(3).to_broadcast([128, ITILES, S, 1])
        t2v = t2b.unsqueeze(1).unsqueeze(3).to_broadcast([128, ITILES, S, 1])

        nc.vector.memset(L[:, :, :, 1:2], 1.0)
        nc.vector.tensor_copy(L[:, :, :, 2:3], tv)
        nc.vector.tensor_copy(L[:, :, :, 3:4], t2v)
        Lx = cpool.tile([128, ITILES, S, 1], F32)
        _tt(nc, Lx[:], b_i(ivx), tv, OP.mult)            # x_i(s) = zx + vx t
        _tt(nc, Lx[:], Lx[:], b_i(izx), OP.add)
        nc.vector.tensor_scalar_mul(L[:, :, :, 4:5], Lx[:], -2.0)
        _tt(nc, L[:, :, :, 5:6], L[:, :, :, 4:5], tv, OP.mult)
        Ly = cpool.tile([128, ITILES, S, 1], F32)
        _tt(nc, Ly[:], b_i(ivy), tv, OP.mult)
        _tt(nc, Ly[:], Ly[:], b_i(izy), OP.add)
        nc.vector.tensor_scalar_mul(L[:, :, :, 6:7], Ly[:], -2.0)
        _tt(nc, L[:, :, :, 7:8], L[:, :, :, 6:7], tv, OP.mult)
        Lr = cpool.tile([128, ITILES, S, 1], F32)
        _tt(nc, L[:, :, :, 0:1], b_i(ib), tv, OP.mult)   # r = a + b t + c t^2
        _tt(nc, L[:, :, :, 0:1], L[:, :, :, 0:1], b_i(ia), OP.add)
        _tt(nc, Lr[:], b_i(ic), t2v, OP.mult)
        _tt(nc, L[:, :, :, 0:1], L[:, :, :, 0:1], Lr[:], OP.add)

        # ---------------- transposes (PE) ----------------
        T2 = cpool.tile([8, N], F32R)                    # G_j rows
        L2 = cpool.tile([8, ITILES * S, 128], F32R)      # F_i(s) rows
        with tc.tile_pool(name="tp", bufs=4, space="PSUM") as tpp:
            for c in range(16):
                pt = tpp.tile([32, 128], F32, tag="pt", name="pt")
                nc.tensor.transpose(pt[:], F[:, c, :], ident)
                nc.vector.tensor_copy(T2[:, c * 128:(c + 1) * 128], pt[0:8, :])
            for it in range(ITILES):
                for s in range(S):
                    pt = tpp.tile([32, 128], F32, tag="pt", name="pt")
                    nc.tensor.transpose(pt[:], L[:, it, s, :], ident)
                    nc.vector.tensor_copy(L2[:, it * S + s, :], pt[0:8, :])

        d_ev = evpool.tile([128, C_EV, 1], F32)

        # ---------------- main pairwise loop ----------------
        sq_insts = [[] for _ in range(ITILES)]
        ex_insts = [[] for _ in range(ITILES)]
        with tc.tile_pool(name="qp", bufs=2, space="PSUM") as qpool, \
                tc.tile_pool(name="wp", bufs=12) as wpool:
            for it in range(ITILES):
                for s in range(S):
                    q = qpool.tile([128, N], F32, tag="q", name="q")
                    for kk in range(4):
                        nc.tensor.matmul(
                            q[:, kk * 512:(kk + 1) * 512],
                            L2[:, it * S + s, :],
                            T2[:, kk * 512:(kk + 1) * 512],
                            start=True, stop=True,
                        )
                    w = wpool.tile([128, N], BF16, tag="w", name="w")
                    nc.vector.tensor_scalar_max(w[:], q[:], 0.0)
                    col = it * S + s
                    sq = nc.scalar.activation(w[:], w[:], AF.Sqrt)
                    ex = nc.scalar.activation(
                        w[:], w[:], AF.Exp, scale=-1.0,
                        accum_out=acc[:, col:col + 1],
                    )
                    sq_insts[it].append(sq)
                    ex_insts[it].append(ex)

            # ---- event tail, at the END of every engine stream ----
            ev_tail = []
            for j in range(SLOT_D):
                emit_dma_event_math(j, wpool)
            ev_tail.append(nc.scalar.activation(
                d_ev[:], d2all[:], AF.Sqrt, accum_out=acc[:, 21:22]
            ))

            # ACT phase order: sqrt(i0) exp(i0) sqrt(i1) exp(i1) ev_g ev_d.
            # The event sqrts land last: their PE/DVE inputs are only
            # ready near the end of the main loop, and must not gate the
            # exp phases.
            order = (
                sq_insts[0] + ex_insts[0] + sq_insts[1] + ex_insts[1]
                + ev_tail
            )
            for a, b in zip(order[1:], order[:-1]):
                add_dep_helper(a.ins, b.ins, reason="act table phase order")

            nc.sync.dma_start(out_p, acc[:])

    nc.compile()
    _CACHE["nc"] = nc
    return nc


def _marshal(inputs):
    z0 = np.asarray(inputs["z0"], dtype=np.float32)
    v0 = np.asarray(inputs["v0"], dtype=np.float32)
    uv = np.asarray(inputs["data_uv"], dtype=np.int32)
    tt = np.asarray(inputs["data_t"], dtype=np.float32)
    t0 = np.float32(np.asarray(inputs["t0"]).reshape(-1)[0])
    tn = np.float32(np.asarray(inputs["tn"]).reshape(-1)[0])

    zv = np.ascontiguousarray(np.concatenate([z0, v0], axis=1)).astype(np.float32)
    dt = np.float32((tn - t0) / np.float32(S))
    tmid = (t0 + (np.arange(S, dtype=np.float32) + np.float32(0.5)) * dt).astype(
        np.float32
    )
    tb = np.ascontiguousarray(np.broadcast_to(tmid, (128, S))).astype(np.float32)
    t2b = (tb * tb).astype(np.float32)

    zv_pad = np.zeros((N, GELEM), np.float32)
    zv_pad[:, 0:4] = zv

    E = uv.shape[0]
    assert E <= NCORES * EV_PER_CORE
    u_all = uv[:, 0].astype(np.int64)
    v_all = uv[:, 1].astype(np.int64)

    def pack_core(u, v, t):
        """Segments of <= SLOT_D events per u-node, sorted by fill count
        (desc) and dealt by global rank to (partition, q).  Returns the
        rank-major index lists (pad suffix = -1), the slot-major t/mask
        planes, and the per-op valid counts."""
        order = np.argsort(u, kind="stable")
        us, vs, ts = u[order], v[order], t[order]
        starts = np.flatnonzero(np.r_[True, us[1:] != us[:-1]])
        ends = np.r_[starts[1:], len(us)]
        segs = []
        for s0, e0 in zip(starts, ends):
            for j in range(s0, e0, SLOT_D):
                segs.append((us[s0], slice(j, min(j + SLOT_D, e0))))
        segs.sort(key=lambda srec: -(srec[1].stop - srec[1].start))
        nseg = len(segs)
        assert nseg <= NSEG, f"{nseg} segments > capacity {NSEG}"

        seg_u = np.full(NSEG, -1, np.int16)
        v_idx = np.full((SLOT_D, NSEG), -1, np.int16)
        t_sl = np.zeros((SLOT_D, NSEG), np.float32)
        m_sl = np.zeros((SLOT_D, NSEG), np.float32)
        for rank, (un, sl) in enumerate(segs):
            ln = sl.stop - sl.start
            seg_u[rank] = un
            v_idx[:ln, rank] = vs[sl]
            t_sl[:ln, rank] = ts[sl]
            m_sl[:ln, rank] = 1.0
        v_cnt = [int((v_idx[j] >= 0).sum()) for j in range(SLOT_D)]
        return seg_u, v_idx, t_sl, m_sl, nseg, v_cnt

    def wrap16(x):
        # rank-major list [NSEG] -> [128, NSEG//16]: idx m at [m%16, m//16],
        # replicated down the 8 blocks of 16 partitions
        w = x.reshape(NSEG // 16, 16).T
        return np.ascontiguousarray(np.tile(w, (8, 1)))

    ident_np = np.eye(128, dtype=np.float32)
    per_core = []
    for k in range(NCORES):
        sl = slice(k * EV_PER_CORE, (k + 1) * EV_PER_CORE)
        per_core.append(pack_core(u_all[sl], v_all[sl], tt[sl]))
    # every core must present exactly reg valid indices per op: round the
    # global max up to 16 and pad shorter cores with masked index 0
    seg_reg = min(NSEG, -(-max(pc[4] for pc in per_core) // 16) * 16)
    v_regs = [
        min(NSEG, -(-max(pc[5][j] for pc in per_core) // 16) * 16)
        for j in range(SLOT_D)
    ]

    in_maps = []
    for k in range(NCORES):
        seg_u, v_idx, t_sl, m_sl, nseg_k, vc_k = per_core[k]
        seg_u[nseg_k:seg_reg] = 0
        for j in range(SLOT_D):
            v_idx[j, vc_k[j]:v_regs[j]] = 0
        zvi = zv[k * 256:(k + 1) * 256]
        # t/mask planes: [128, SLOT_D, SPD]: [p, j, q] = slot j of the
        # segment at rank q*128+p
        t_pl = t_sl.reshape(SLOT_D, SPD, 128).transpose(2, 0, 1).reshape(128, C_EV)
        m_pl = m_sl.reshape(SLOT_D, SPD, 128).transpose(2, 0, 1).reshape(128, C_EV)
        fblob = np.concatenate(
            [
                zv.reshape(16, 128, 4).transpose(1, 0, 2).reshape(128, 64),
                zvi.reshape(2, 128, 4).transpose(1, 0, 2).reshape(128, 8),
                tb,
                t2b,
                ident_np,
                np.zeros((128, 8), np.float32),
                t_pl,
                m_pl,
            ],
            axis=1,
        ).astype(np.float32)
        iblob = np.concatenate(
            [wrap16(seg_u)] + [wrap16(v_idx[j]) for j in range(SLOT_D)],
            axis=1,
        ).astype(np.int16)
        in_maps.append(
            {
                "zv_pad": zv_pad,
                "fblob": np.ascontiguousarray(fblob),
                "iblob": np.ascontiguousarray(iblob),
            }
        )
    return in_maps, (float(t0), float(tn), E), (seg_reg, v_regs)


def _np_event_total(inputs, core):
    """float64 reference event-distance sum for one core's slice."""
    z0 = np.asarray(inputs["z0"], np.float64)
    v0 = np.asarray(inputs["v0"], np.float64)
    uv = np.asarray(inputs["data_uv"], np.int64)
    tt = np.asarray(inputs["data_t"], np.float64)
    sl = slice(core * EV_PER_CORE, (core + 1) * EV_PER_CORE)
    u, v, t = uv[sl, 0], uv[sl, 1], tt[sl]
    dx = (z0[u, 0] - z0[v, 0]) + (v0[u, 0] - v0[v, 0]) * t
    dy = (z0[u, 1] - z0[v, 1]) + (v0[u, 1] - v0[v, 1]) * t
    return np.sqrt(dx * dx + dy * dy).sum()


def _combine(core_outs, beta, t0, tn, E):
    """core_outs: list of [128, 24] float32 partial-sum tensors."""
    exp_sum = 0.0
    ev_sum = 0.0
    for o in core_outs:
        o = np.asarray(o, dtype=np.float64)
        exp_sum += o[:, 0 : ITILES * S].sum()
        ev_sum += o[:, 20].sum() + o[:, 21].sum()
    b = float(beta)
    dt = (tn - t0) / S
    event_intensity = E * b - ev_sum
    non_event = np.exp(b) * (exp_sum - S * N) / 2.0 * dt
    return np.float32(event_intensity - 1.0 * non_event)


def kernel(**inputs) -> np.ndarray:
    from concourse.bass_utils import run_bass_kernel_spmd

    in_maps, (t0, tn, E), regs = _marshal(inputs)
    nc = _build(regs)
    res = run_bass_kernel_spmd(nc, in_maps, core_ids=list(range(NCORES)))
    beta = float(np.asarray(inputs["beta"]).reshape(-1)[0])
    out = _combine([r["out_p"] for r in res.results], beta, t0, tn, E)
    return np.asarray(out, dtype=np.float32)
